# revision 56
# baseline (speedup 1.0000x reference)
"""Trainium2 Bass kernel for nn_AttributeAttn (dense_transformer, memory-bound).

Math (collapsed reference):
    u = W.T @ v; uh, ue = u[:H], u[H:]
    hv[n,b] = hidden[n,b,:] @ uh          # the big reduction
    ev[c,b] = enc[c,b,:] @ ue
    bias    = b @ v
    out[b,n,c] = softmax_c(tanh(hv[n,b] + ev[c,b] + bias))

Distribution: data-parallel over B (4 batches per core, 8 cores).

HBM-streaming bound: ~11.8MB/core of bf16 inputs against a measured
~420GB/s two-queue DMA aggregate (16 engines x ~26GB/s; engine E79 runs
~13% slow and sets each transfer's completion).  Trace-driven design:

  - All inputs bf16 (2e-2 rel-err gate leaves ~2.4x slack at 8.2e-3).
  - Two HWDGE rings: sync [vb+jb0, jb1, h0, h2, h4, h7a], scalar
    [jb2, enc, h1, h3, h5, h6, h7b]; blocks are processed in arrival
    order with each block's softmax chain emitted immediately after
    its contraction.  The last block is split 3+1: 7a=bb012 rides
    sync mid-stream, 7b=bb3 (0.26MB) ends the scalar stream, so the
    tail holds one QUARTER-width chain (~1.4us serial: tanh, then exp
    emits its row-sum via ACT accum_out -- no DVE reduce or its
    cross-engine wait -- then recip + mul).
  - Each score PSUM tile is SEEDED with the ev broadcast (ones x
    ev_row matmul, start=True) and the hv rank-1s accumulate on top
    (start=False) -- the separate DVE add disappears and tanh reads
    PSUM directly.  Chain = tanh(+bias) -> exp -> reduce -> recip ->
    normalize-mul, bf16 from tanh onward; DVE and ACT are both ~1us
    per chain, the chain cadence at the stream tail.
  - hv row copies ride DVE; output leaves in four stores: gpsimd
    (SWDGE) flushes blocks 0-3 and 4-5 mid-stream, sync flushes 6+7a
    while chain(7b) runs, and the final store is only 7b's 33KB.
  - f32 warm-up matmuls at the head, 2 bf16 dummies after each block,
    and 6 before the final half-block hold the PE HAM clock at 2.4GHz.
    HAM works in 3.41us quantized duty windows; mid-stream PE util is
    ~75%, right at the ~68% gate, so without filler a window randomly
    trips to 1.2GHz and costs ~4us downstream -- this is the dominant
    run-to-run variance (bimodal ~53 vs ~56-58us; more filler than
    this made it WORSE, as did warm bursts before c(6)).

Measured/rejected: DRAM-bounce hv transposes (+17us); W sharded
across cores + AllGather of u (~55us collective); fp8 hidden (~3e-2
rel err, over the gate); 15-row partition-sliced tail DMAs to dodge
the slow E79 engine (per-transfer overhead ~0.5us made the tail 4x
slower); a jb1 load on the gpsimd SWDGE queue (wrong results + only
122GB/s); tc.high_priority() on the chains, both blanket and
selective-early-blocks (+3-4us each time -- ANY priority perturbation
slows the stream and never actually hoists the rank-1s; the
scheduler's back-loading protects the in-order engines); merged
jb2+enc transfer (10KB packets slowed the W phase); gpsimd
tensor_reduce (C-axis only) and gpsimd scalar_tensor_tensor (runtime
crash); DVE scalar_tensor_tensor with op1=divide to fuse away the
reciprocal (AluOpType.divide exists and the interpreter supports it,
but neuronxcc lowering crashes -- hardware DVE has no divide).  DMA facts: dma_starts fan out round-robin over 16 engines at
~26GB/s each, two queues sustain ~420GB/s aggregate; E79 runs ~13%
slow, so the global last-byte lands at ~43us REGARDLESS of queue
balance (measured invariant across 5 ring splits) -- do not bother
rebalancing to chase it; each extra transfer costs ~0.3-0.5us of
queue gap; kernel preamble ~6.7us and teardown ~2.7us are fixed.
The device also drifts between fast/slow regimes across minutes;
never conclude from fewer than ~4 samples.

Host side only shards/transposes/casts (no module math on host).
"""
import sys
import types

import numpy as np
import ml_dtypes

BF = ml_dtypes.bfloat16

# The container's antenv stub lacks axon_hooks; provide it so trace=True
# works when the test harness requests profiling. Harmless otherwise.
if "antenv.axon_hooks" not in sys.modules:
    _hooks_mod = types.ModuleType("antenv.axon_hooks")
    try:
        from trn_agent_boot.trn_boot import _ntff_profile_via_ctypes
        _ntff_hook = _ntff_profile_via_ctypes("/opt/axon/libaxon_pjrt.so")
    except Exception:
        _ntff_hook = None
    _hooks_mod.get_axon_ntff_profile_hook = lambda: _ntff_hook
    _hooks_mod.set_axon_ntff_profile_hook = lambda h: None
    sys.modules["antenv.axon_hooks"] = _hooks_mod

import concourse.bacc as bacc
import concourse.tile as tile
from concourse import mybir
from concourse.bass_utils import run_bass_kernel_spmd

f32 = mybir.dt.float32
bf16 = mybir.dt.bfloat16
AF = mybir.ActivationFunctionType
X = mybir.AxisListType.X
ADD = mybir.AluOpType.add
MUL = mybir.AluOpType.mult

N, B, H = 1024, 32, 1024
C, K = 64, 512
NCORES = 8
BPC = B // NCORES            # 4 batches per core
HC = H // 128                # 8 h-chunks
KC = K // 128                # 4 k-chunks
JC = (H + K) // 128          # 12 u columns
NBLK = N // 128              # 8 n-blocks per core
FW = BPC * C                 # 256 free (bb, c) elements per n-block
BW = 128 * BPC               # 512 hv free elements per n-block
WROW = H + K                 # 1536
ENC_W = KC * FW              # 1024 enc columns per partition
HB = NBLK * HC * BW          # hid columns per partition
WV_W = 2 * HC + HC * WROW + ENC_W   # vb | w chunks 0..7 | enc

SYNC_BLKS = [0, 2, 4]        # full blocks on the sync ring
SCAL_BLKS = [1, 3, 5, 6]     # full blocks on the scalar ring
# processing order, matched to measured arrivals (h5 lands just before
# h7a).  Block 7 is split 3+1: 7a=bb012 on sync lands mid-stream,
# 7b=bb3 (0.26MB) ends the scalar stream with the narrowest possible
# final chain.
ORDER = [0, 1, 2, 3, 4, 5, "7a", 6, "7b"]
NB7A, NB7B = 3, 1            # bb widths of the block-7 pieces

# Set by test harness to capture an NTFF profile.
TRACE = False
TRACE_KW = {}
LAST_RESULT = None

_cached = None


def _build():
    nc = bacc.Bacc(None, target_bir_lowering=False)
    wv_d = nc.dram_tensor("wv", [128, WV_W], bf16, kind="ExternalInput")
    hid_d = nc.dram_tensor("hid", [128, HB], bf16, kind="ExternalInput")
    out_d = nc.dram_tensor("out", [128, NBLK * FW], bf16, kind="ExternalOutput")

    with tile.TileContext(nc) as tc:
        with (
            tc.tile_pool(name="consts", bufs=1) as consts,
            tc.tile_pool(name="work", bufs=3) as work,
            tc.tile_pool(name="ps_warm", bufs=1, space="PSUM") as pw,
        ):
            # --- loads.  W is stored j-major (jb blocks of 512 u-columns,
            # ic-minor).  sync leads with the hv-critical half (vb+jb0,
            # then jb1); scalar carries jb2+enc as ONE transfer so its
            # ring reaches hid sooner.
            wv_sb = consts.tile([128, WV_W], bf16, tag="wv")
            mid0 = 2 * HC + HC * 512
            mid = 2 * HC + 2 * HC * 512
            mid2 = mid + HC * 512
            nc.sync.dma_start(out=wv_sb[:, :mid0], in_=wv_d[:, :mid0])
            nc.sync.dma_start(out=wv_sb[:, mid0:mid], in_=wv_d[:, mid0:mid])
            nc.scalar.dma_start(out=wv_sb[:, mid:mid2], in_=wv_d[:, mid:mid2])
            nc.scalar.dma_start(out=wv_sb[:, mid2:], in_=wv_d[:, mid2:])
            vb_sb = wv_sb[:, 0:2 * HC]
            enc_sb = wv_sb[:, 2 * HC + HC * WROW:]

            def wjb(jb, ic):
                off = 2 * HC + (jb * HC + ic) * 512
                return wv_sb[:, off:off + 512]

            # hid tiles: full blocks 0-6, block 7 as two bb-halves.
            hid_sb = {}
            for k in SYNC_BLKS:
                t = consts.tile([128, HC * BW], bf16, tag=f"hid{k}")
                nc.sync.dma_start(
                    out=t, in_=hid_d[:, k * HC * BW:(k + 1) * HC * BW])
                hid_sb[k] = t
            for k in SCAL_BLKS:
                t = consts.tile([128, HC * BW], bf16, tag=f"hid{k}")
                nc.scalar.dma_start(
                    out=t, in_=hid_d[:, k * HC * BW:(k + 1) * HC * BW])
                hid_sb[k] = t
            W7A = HC * 128 * NB7A
            h7a = consts.tile([128, W7A], bf16, tag="hid7a")
            nc.sync.dma_start(
                out=h7a, in_=hid_d[:, 7 * HC * BW:7 * HC * BW + W7A])
            h7b = consts.tile([128, HC * 128 * NB7B], bf16, tag="hid7b")
            # (an hc-halved h7b transfer to pre-gate the tail matmuls
            # measured ~0.6us WORSE -- the extra transfer's queue gap
            # outweighs the ~0.36us earlier first-half completion)
            nc.scalar.dma_start(
                out=h7b, in_=hid_d[:, 7 * HC * BW + W7A:8 * HC * BW])

            # --- PE warm-up (holds HAM clock while W streams in)
            warm_src = consts.tile([128, 512], f32, tag="warm_src")
            nc.vector.memset(warm_src, 1.0)
            warm_bf = consts.tile([128, 512], bf16, tag="warm_bf")
            nc.vector.tensor_copy(warm_bf, warm_src)
            ones_f = consts.tile([1, 128], f32, tag="ones_f")
            nc.vector.memset(ones_f, 1.0)
            ones = consts.tile([1, 128], bf16, tag="ones")
            nc.vector.tensor_copy(ones, ones_f)
            warm_ps = pw.tile([1, 512], f32, tag="warm")

            def warm(n):
                for _ in range(n):
                    nc.tensor.matmul(warm_ps, warm_src[:, 0:1], warm_src,
                                     start=True, stop=True)

            def warm_fast(n):
                for _ in range(n):
                    nc.tensor.matmul(warm_ps, warm_bf[:, 0:1], warm_bf,
                                     start=True, stop=True,
                                     skip_group_check=True)

            warm(2)

            with tc.tile_pool(name="ps_setup", bufs=1, space="PSUM") as pset:
                # u row = v.T @ W (1, 1536), jb-major.  jb0/jb1 (the uh
                # half that gates the hv path) ride sync and complete
                # first; uh columns are built BEFORE the ev path so
                # contraction(0) is never blocked on scalar's W piece.
                u_ps = pset.tile([1, 3, 512], f32, tag="u")
                bias_ps = pset.tile([1, FW], f32, tag="m", bufs=2,
                                    name="bias")
                u_row = consts.tile([1, JC, 128], bf16, tag="urow")
                urf = u_row.rearrange("p a b -> p (a b)")

                def ugroup(jb):
                    for ic in range(HC):
                        nc.tensor.matmul(
                            u_ps[:, jb, :], vb_sb[:, ic:ic + 1],
                            wjb(jb, ic),
                            start=(ic == 0), stop=(ic == HC - 1))

                ugroup(0)
                # bias = b @ v (8 rank-1s, ~1 cycle each)
                for icb in range(HC):
                    nc.tensor.matmul(
                        bias_ps[:, 0:1], vb_sb[:, icb:icb + 1],
                        vb_sb[:, HC + icb:HC + icb + 1],
                        start=(icb == 0), stop=(icb == HC - 1))
                nc.vector.tensor_copy(
                    urf[:, 0:512].rearrange("p (x y) -> p x y", x=1),
                    u_ps[:, 0:1, :])
                ugroup(1)
                nc.scalar.copy(urf[:, 512:1024], u_ps[:, 1, :])
                bias_sb = consts.tile([1, 1], bf16, tag="bias_sb")
                nc.vector.tensor_copy(bias_sb, bias_ps[:, 0:1])

                # uh columns (128, 8) for the hv contraction -- needs
                # only jb0/jb1, so it runs before/while jb2 lands.
                uch_ps = pset.tile([128, HC], f32, tag="uc", bufs=2,
                                   name="uch")
                for jc in range(HC):
                    nc.tensor.matmul(
                        uch_ps[:, jc:jc + 1], u_row[0:1, jc, :],
                        ones[:, 0:1], start=True, stop=True)
                ucols = consts.tile([128, HC], bf16, tag="ucols")
                nc.vector.tensor_copy(ucols, uch_ps)

                ugroup(2)
                nc.vector.tensor_copy(
                    urf[:, 1024:1536].rearrange("p (x y) -> p x y", x=1),
                    u_ps[:, 2:3, :])

                # ue columns (128, 4) -> ev path
                uce_ps = pset.tile([128, KC], f32, tag="uc", bufs=2,
                                   name="uce")
                for kc in range(KC):
                    nc.tensor.matmul(
                        uce_ps[:, kc:kc + 1], u_row[0:1, HC + kc, :],
                        ones[:, 0:1], start=True, stop=True)
                ucols_e = consts.tile([128, KC], bf16, tag="ucols_e")
                nc.vector.tensor_copy(ucols_e, uce_ps)

                ev_ps = pset.tile([1, FW], f32, tag="m", bufs=2, name="ev")
                for kc in range(KC):
                    nc.tensor.matmul(
                        ev_ps, ucols_e[:, kc:kc + 1],
                        enc_sb[:, kc * FW:(kc + 1) * FW],
                        start=(kc == 0), stop=(kc == KC - 1))
                ev_row = consts.tile([1, FW], bf16, tag="ev_row")
                nc.vector.tensor_copy(ev_row, ev_ps)

                # ev broadcast to all partitions; bias broadcast column
                bcol_ps = pset.tile([128, 1], f32, tag="m", bufs=2,
                                    name="bcol")
                nc.tensor.matmul(bcol_ps, ones, bias_sb, start=True,
                                 stop=True)
                bias_col = consts.tile([128, 1], f32, tag="bias_col")
                nc.vector.tensor_copy(bias_col, bcol_ps)
                warm(1)

            # --- per block: contract over H -> [1, n*bb] psum row; DVE
            # copies it to SBUF; PE rank-1s broadcast it across the 128
            # n-partitions; fused add+tanh(+bias)+exp+reduce+normalize.
            # Chains run immediately after their contraction, in
            # arrival order, so the tail holds exactly one half-chain.
            o_all = consts.tile([128, NBLK * FW], bf16, tag="o_all")
            with tc.tile_pool(name="ps_main", bufs=1, space="PSUM") as pp:
                rows = {}

                def contract(key, src, nbb):
                    # nbb = bb-width (4 for full blocks, 2 for halves)
                    w = 128 * nbb
                    acc = pp.tile([1, BW], f32, tag="acc", bufs=3,
                                  name=f"acc_{key}")[:, :w]
                    for hc in range(HC):
                        nc.tensor.matmul(
                            acc, ucols[:, hc:hc + 1],
                            src[:, hc * w:(hc + 1) * w],
                            start=(hc == 0), stop=(hc == HC - 1))
                    return acc

                def rowcopy(key, acc, nbb):
                    row = work.tile([1, BW], bf16, tag="row", bufs=3,
                                    name=f"row_{key}")[:, :128 * nbb]
                    nc.vector.tensor_copy(row, acc)
                    rows[key] = row

                # the final chain's ev seed depends only on ev_row
                # (ready ~19us), not on h7b -- pre-seed a dedicated
                # PSUM tile early so the tail's PE path is a single
                # rank-1 between data arrival and tanh.
                sc7b_ps = pp.tile([128, NB7B * C], f32, tag="sc7b",
                                  bufs=1)
                nc.tensor.matmul(
                    sc7b_ps, ones, ev_row[:, NB7A * C:FW],
                    start=True, stop=False, skip_group_check=True)
                # chain(6) is also tail-critical when h6's straggler
                # packets land late; pre-seed its score bank too.
                sc6_ps = pp.tile([128, FW], f32, tag="sc6", bufs=1)
                nc.tensor.matmul(
                    sc6_ps, ones, ev_row,
                    start=True, stop=False, skip_group_check=True)

                def rank1s(key, nbb, evb0):
                    # seed the score PSUM with the ev broadcast (ones x
                    # ev_row), then accumulate the hv rank-1s on top --
                    # the separate DVE add disappears and tanh reads
                    # PSUM directly.
                    rowv = rows[key].rearrange("p (n bb) -> p bb n", bb=nbb)
                    if key == "7b":
                        nc.tensor.matmul(
                            sc7b_ps, rowv[0:1, 0, :], ones[:, 0:C],
                            start=False, stop=True,
                            skip_group_check=True)
                        return sc7b_ps
                    if key == 6:
                        for bb in range(nbb):
                            nc.tensor.matmul(
                                sc6_ps[:, bb * C:(bb + 1) * C],
                                rowv[0:1, bb, :], ones[:, 0:C],
                                start=False, stop=(bb == nbb - 1),
                                skip_group_check=True)
                        return sc6_ps
                    sc_ps = pp.tile([128, FW], f32, tag="score", bufs=2,
                                    name=f"score_{key}")[:, :nbb * C]
                    nc.tensor.matmul(
                        sc_ps, ones, ev_row[:, evb0:evb0 + nbb * C],
                        start=True, stop=False, skip_group_check=True)
                    for bb in range(nbb):
                        nc.tensor.matmul(
                            sc_ps[:, bb * C:(bb + 1) * C],
                            rowv[0:1, bb, :], ones[:, 0:C],
                            start=False, stop=(bb == nbb - 1),
                            skip_group_check=True)
                    return sc_ps

                def chain(key, sc_ps, nbb, ocol0):
                    # tanh reads the seeded PSUM directly; bf16 from
                    # there on (tanh output spans [-1,1], bf16 eps
                    # 0.004 there, well inside the 2e-2 gate).
                    w = nbb * C
                    sc = work.tile([128, FW], bf16, tag="sc", bufs=2,
                                   name=f"sc_{key}")[:, :w]
                    nc.scalar.activation(out=sc, in_=sc_ps, func=AF.Tanh,
                                         bias=bias_col)
                    den = work.tile([128, BPC], f32, tag="den", bufs=2,
                                    name=f"den_{key}")[:, :nbb]
                    sc3 = sc.rearrange("p (bb c) -> p bb c", c=C)
                    if nbb == 1:
                        # single-bb chain: exp emits its row-sum via
                        # accum_out in the same ACT pass, removing the
                        # DVE reduce (and its wait) from the tail's
                        # serial path.
                        nc.scalar.activation(out=sc, in_=sc, func=AF.Exp,
                                             accum_out=den)
                    else:
                        nc.scalar.activation(out=sc, in_=sc, func=AF.Exp)
                        nc.vector.tensor_reduce(den, sc3, axis=X, op=ADD)
                    nc.vector.reciprocal(den, den)
                    o3 = o_all[:, ocol0:ocol0 + w].rearrange(
                        "p (bb c) -> p bb c", c=C)
                    nc.vector.scalar_tensor_tensor(
                        out=o3, in0=sc3, scalar=1.0,
                        in1=den[:, :, None].broadcast_to([128, nbb, C]),
                        op0=MUL, op1=MUL)

                # per block, in predicted arrival order: contraction,
                # DVE row copy, PE rank-1 broadcast, softmax chain --
                # all immediately, so chains can never stack at the
                # tail.  acc bufs=3 bounds how far the scheduler can
                # hoist later contractions past a pending chain.
                for key in ORDER:
                    if key == "7a":
                        src, nbb, evb0, oc0 = h7a, NB7A, 0, 7 * FW
                    elif key == "7b":
                        src, nbb, evb0, oc0 = (h7b, NB7B, NB7A * C,
                                               7 * FW + NB7A * C)
                    else:
                        src, nbb, evb0, oc0 = hid_sb[key], BPC, 0, key * FW
                    # noqa: evb0 selects the (bb,c) slice of ev_row
                    if key == "7b":
                        # dummy matmuls bridge the PE idle gap while
                        # h7b's last packets land, holding 2.4GHz.
                        warm_fast(6)
                    acc = contract(key, src, nbb)
                    rowcopy(key, acc, nbb)
                    sc_ps = rank1s(key, nbb, evb0)
                    chain(key, sc_ps, nbb, oc0)
                    if key not in (0, "7b"):
                        # mid-stream PE utilization sits at ~75%, right
                        # at the HAM throttle threshold (3.41us duty
                        # windows); idle gaps between data-gated
                        # contractions randomly trip 1.2GHz windows
                        # that stretch contractions 1.7x (the main
                        # run-to-run variance).  Cheap bf16 dummies
                        # per block keep utilization above the gate;
                        # heavier doses (4/block, bursts before c(6) or
                        # before c(0)) measured WORSE every time tried.
                        warm_fast(2)
                    if key == 3:
                        nc.gpsimd.dma_start(
                            out=out_d[:, 0:4 * FW], in_=o_all[:, 0:4 * FW])
                    elif key == 5:
                        # flush 4-5 mid-stream
                        nc.gpsimd.dma_start(
                            out=out_d[:, 4 * FW:6 * FW],
                            in_=o_all[:, 4 * FW:6 * FW])
                    elif key == 6:
                        # 6 + 7a leave while chain(7b) runs; the final
                        # store is only 7b's 33KB quarter.
                        nc.sync.dma_start(
                            out=out_d[:, 6 * FW:7 * FW + NB7A * C],
                            in_=o_all[:, 6 * FW:7 * FW + NB7A * C])
                nc.sync.dma_start(
                    out=out_d[:, 7 * FW + NB7A * C:],
                    in_=o_all[:, 7 * FW + NB7A * C:])
    nc.compile()
    return nc


def kernel(**inputs):
    global _cached, LAST_RESULT
    hidden = np.asarray(inputs["hidden"], dtype=np.float32)
    enc = np.asarray(inputs["encoder_outputs"], dtype=np.float32)
    W = np.asarray(inputs["W"], dtype=np.float32)
    b = np.asarray(inputs["b"], dtype=np.float32)
    v = np.asarray(inputs["v"], dtype=np.float32)

    if _cached is None:
        _cached = _build()
    nc = _cached

    # vb: column ic holds v[ic*128:(ic+1)*128]; column HC+ic holds b chunk.
    vb = np.concatenate(
        [v.reshape(HC, 128).T, b.reshape(HC, 128).T], axis=1).astype(BF)
    # W j-major: wv[p, 16 + (jb*HC + ic)*512 + jj] = W[ic*128+p, jb*512+jj]
    wt = W.astype(BF).reshape(HC, 128, WROW).transpose(1, 0, 2)
    wj = wt.reshape(128, HC, 3, 512).transpose(0, 2, 1, 3)
    wv_head = np.concatenate([vb, wj.reshape(128, HC * WROW)], axis=1)

    hb = hidden.astype(BF)
    eb = enc.astype(BF)

    in_maps = []
    for j in range(NCORES):
        bsl = slice(j * BPC, (j + 1) * BPC)
        # hid: [p, (k, hc, n*BPC+bb)] for blocks 0-6; block 7 is stored
        # as two bb-halves [p, (half, hc, n*2+bbh)].
        x = hb[:, bsl, :]                                   # (N, BPC, H)
        x = x.transpose(2, 0, 1)                            # (H, N, BPC)
        x = x.reshape(HC, 128, NBLK, 128, BPC)              # hc p k n bb
        full = x[:, :, :7].reshape(HC, 128, 7, BW)
        full = full.transpose(1, 2, 0, 3).reshape(128, -1)  # p (k hc f)
        b7 = x[:, :, 7]                                     # hc p n bb
        b7a = b7[:, :, :, :NB7A].transpose(1, 0, 2, 3)      # p hc n bb
        b7a = b7a.reshape(128, -1)                          # p (hc n bb)
        b7b = b7[:, :, :, NB7A:].transpose(1, 0, 2, 3)
        b7b = b7b.reshape(128, -1)
        hid_t = np.ascontiguousarray(
            np.concatenate([full, b7a, b7b], axis=1))
        # enc: [p, kc*FW + bb*C + c]
        e = eb[:, bsl, :].transpose(2, 1, 0)                # (K, BPC, C)
        e = e.reshape(KC, 128, FW).transpose(1, 0, 2)
        enc_t = e.reshape(128, ENC_W)
        wv = np.ascontiguousarray(np.concatenate([wv_head, enc_t], axis=1))
        in_maps.append({"hid": hid_t, "wv": wv})

    res = run_bass_kernel_spmd(
        nc, in_maps, core_ids=list(range(NCORES)), trace=TRACE, **TRACE_KW)
    LAST_RESULT = res

    out = np.empty((B, N, C), dtype=np.float32)
    for j in range(NCORES):
        o = res.results[j]["out"].astype(np.float32)
        o = o.reshape(128, NBLK, BPC, C).transpose(2, 1, 0, 3)
        out[j * BPC:(j + 1) * BPC] = o.reshape(BPC, N, C)
    return out
